# revision 16
# baseline (speedup 1.0000x reference)
"""DirectedDualSAGE (2-layer dual-direction GraphSAGE + MLP head) on 8 trn2
NeuronCores via Bass/Tile.

Strategy (dst-sharded):
- Nodes block-sharded 6250/core; each core owns all edges whose dst falls in
  its shard (both directions).
- Per layer, the SAGE "lin_l(mean(x_j))" term is computed as
  diag(1/cnt) * A * (x @ Wl): transform first (64-dim), then aggregate, which
  halves gather traffic. Aggregation = per-edge row gather (dma_gather) of the
  transformed feature table + segmented accumulation on the vector engine
  using degree-sorted "rounds" (round r adds the r-th edge of every dst; after
  sorting dsts by descending degree each round is a prefix, so the add is one
  contiguous tensor_tensor op).
- Edge srcs span all nodes but dma_gather indices are int16, so the feature
  table is split in two halves (src < 25088 / >= 25088) with per-half streams,
  accumulators and zero-pad rows.
- Layer 1's table (x @ [Wl_in|Wl_out]) is computed replicated on every core
  (x is an input, so no communication). Layer 2's table is computed from the
  local x2 shard and AllGather'ed.
- All dense math runs feature-major on the tensor engine; the aggregated means
  (node-major from the gather) are transposed back via PE identity-matmuls that
  accumulate into the same PSUM as the x @ Wr term.

kernel(**inputs) takes the full unsharded inputs and returns the full [N]
output; sharding happens inside.
"""

import numpy as np

import concourse.bacc as bacc
import concourse.tile as tile
import concourse.mybir as mybir
from concourse import bass_utils

F32 = mybir.dt.float32
I16 = mybir.dt.int16

N = 50000
NC = 8
NLOC = N // NC            # 6250
NLOCP = 6272              # 49*128
NCH = NLOCP // 128        # 49 chunks
NFULLP = 50048            # 391*128
HALF = 25088              # src half split
ZHEAD = 128               # zero rows at table head
TROWS = ZHEAD + NFULLP + 128   # 50304
BOFF = ZHEAD + HALF       # 25216, table-B base row
BROWS = TROWS - BOFF      # 25088
BZERO = ZHEAD + NFULLP - BOFF  # 24960 -> row 50176 (zero tail)
SMAX = 6400               # max rows per dma_gather call

_CACHE = {}
DEBUG = False


# ----------------------------------------------------------------- host prep

def _round_up(v, m):
    return (v + m - 1) // m * m


def _per_core_half(src, dst, half_mask):
    out = []
    for c in range(NC):
        m = (dst // NLOC == c) & half_mask
        s = src[m]
        dloc = (dst[m] - c * NLOC).astype(np.int64)
        deg = np.bincount(dloc, minlength=NLOCP).astype(np.int64)
        perm = np.argsort(-deg, kind="stable").astype(np.int64)
        pos = np.empty(NLOCP, dtype=np.int64)
        pos[perm] = np.arange(NLOCP)
        order = np.argsort(dloc, kind="stable")
        sd = dloc[order]
        ss = s[order]
        if len(sd):
            starts = np.r_[0, 1 + np.flatnonzero(np.diff(sd))]
            group_id = np.zeros(len(sd), dtype=np.int64)
            group_id[starts[1:]] = 1
            group_id = np.cumsum(group_id)
            rank = np.arange(len(sd)) - starts[group_id]
        else:
            rank = sd
        slot = pos[sd]
        maxdeg = int(deg.max()) if len(sd) else 0
        rounds = []
        for r in range(maxdeg):
            mr = rank == r
            rounds.append((int(np.count_nonzero(mr)), slot[mr], ss[mr]))
        out.append(dict(deg=deg, pos=pos, rounds=rounds))
    return out


def _preprocess(edge_index_in, edge_index_out):
    plan = {"dirs": {}}
    for dname, ei in (("in", edge_index_in), ("out", edge_index_out)):
        src = ei[0].astype(np.int64)
        dst = ei[1].astype(np.int64)
        dinfo = {"halves": {}, "recip": []}
        for c in range(NC):
            m = dst // NLOC == c
            dloc = dst[m] - c * NLOC
            cnt = np.bincount(dloc, minlength=NLOCP).astype(np.float32)
            dinfo["recip"].append((1.0 / np.maximum(cnt, 1.0)).astype(np.float32))
        for hname, is_a in (("A", True), ("B", False)):
            half_mask = (src < HALF) if is_a else (src >= HALF)
            cores = _per_core_half(src, dst, half_mask)
            nrounds = max(len(ci["rounds"]) for ci in cores)
            NR = []
            for r in range(nrounds):
                mx = max((ci["rounds"][r][0] if r < len(ci["rounds"]) else 0)
                         for ci in cores)
                NR.append(_round_up(max(mx, 1), 128))
            NR[0] = NLOCP  # full first round: copy-initializes the accumulator
            zi = 0 if is_a else BZERO
            streams = []
            for ci in cores:
                parts = []
                for r in range(nrounds):
                    vec = np.full(NR[r], zi, dtype=np.int64)
                    if r < len(ci["rounds"]):
                        _, slots, ss = ci["rounds"][r]
                        vec[slots] = (ss + ZHEAD) if is_a else (ss - HALF)
                    parts.append(vec)
                stream = np.concatenate(parts)
                assert stream.max(initial=0) < 32768
                streams.append(stream.astype(np.int16))
            groups = []
            cur, cur_rows, off = [], 0, 0
            for r in range(nrounds):
                if cur and cur_rows + NR[r] > SMAX:
                    groups.append((off, cur))
                    off += cur_rows
                    cur, cur_rows = [], 0
                cur.append((cur_rows, NR[r], r))
                cur_rows += NR[r]
            if cur:
                groups.append((off, cur))
            dinfo["halves"][hname] = dict(
                NR=NR, L=int(sum(NR)), streams=streams, groups=groups,
                unperm=[ci["pos"].astype(np.int16) for ci in cores], is_a=is_a,
            )
        plan["dirs"][dname] = dinfo
    return plan


def _wrap_idx(idx):
    L = idx.shape[0]
    assert L % 16 == 0
    w = idx.reshape(L // 16, 16).T.astype(np.int16)
    return np.ascontiguousarray(np.tile(w, (8, 1)))


# ------------------------------------------------------------- device program

def _build_program(plan):
    nc = bacc.Bacc("TRN2", target_bir_lowering=False, debug=False,
                   num_devices=NC)
    dims = ("in", "out")
    inp = {}

    def dram_in(name, shape, dt=F32):
        inp[name] = nc.dram_tensor(name, list(shape), dt, kind="ExternalInput")
        return inp[name]

    xt_full = dram_in("xt_full", [128, NFULLP])
    xt_loc = dram_in("xt_loc", [128, NLOCP])
    ident = dram_in("ident", [128, 128])
    for li in (1, 2):
        dram_in(f"wl_comb{li}", [128, 128])
        dram_in(f"wr_in{li}", [128, 64])
        dram_in(f"wr_out{li}", [128, 64])
        dram_in(f"bias_pk{li}", [128, 1])
        dram_in(f"wcx{li}", [128, 128])
        dram_in(f"wch{li}", [128, 128])
        dram_in(f"cb{li}", [128, 1])
    dram_in("fw", [128, 1])
    dram_in("fb", [1, 1])
    for d in dims:
        dram_in(f"recip_{d}", [128, NCH])
        for h in ("A", "B"):
            L = plan["dirs"][d]["halves"][h]["L"]
            dram_in(f"stream_{d}_{h}", [128, L // 16], I16)
            dram_in(f"unperm_{d}_{h}", [128, NLOCP // 16], I16)

    y1tab = nc.dram_tensor("y1tab", [TROWS, 128], F32, kind="Internal")
    y2tab = nc.dram_tensor("y2tab", [TROWS, 128], F32, kind="Internal",
                           addr_space="Shared")
    y2slice = nc.dram_tensor("y2slice", [NLOCP, 128], F32, kind="Internal")
    accd = {d: nc.dram_tensor(f"accd_{d}", [2, NLOCP, 64], F32, kind="Internal")
            for d in dims}
    out_t = nc.dram_tensor("out", [1, NLOC], F32, kind="ExternalOutput")
    dbg = {}
    if DEBUG:
        for nm, shp in (("dbg_mean_in1", [NLOCP, 64]), ("dbg_mean_out1", [NLOCP, 64]),
                        ("dbg_h1", [128, NLOCP]), ("dbg_x2", [128, NLOCP]),
                        ("dbg_accA", [NLOCP, 64]), ("dbg_accB", [NLOCP, 64]),
                        ("dbg_natA", [NLOCP, 64]), ("dbg_natB", [NLOCP, 64]),
                        ("dbg_stgB", [12 * SMAX, 64])):
            dbg[nm] = nc.dram_tensor(nm, shp, F32, kind="ExternalOutput")

    RELU = mybir.ActivationFunctionType.Relu
    COPY = mybir.ActivationFunctionType.Copy

    with tile.TileContext(nc) as tc:
        with tc.tile_pool(name="const", bufs=1) as cpool, \
             tc.tile_pool(name="idxp", bufs=1) as idxp, \
             tc.tile_pool(name="idxg", bufs=3) as idxgp, \
             tc.tile_pool(name="feat", bufs=2) as featp, \
             tc.tile_pool(name="accp", bufs=2) as accp, \
             tc.tile_pool(name="meanp", bufs=2) as meanp, \
             tc.tile_pool(name="stg", bufs=3) as stgp, \
             tc.tile_pool(name="ybld", bufs=3) as ybldp, \
             tc.tile_pool(name="ystg", bufs=2) as ystgp, \
             tc.tile_pool(name="small", bufs=1) as smallp, \
             tc.tile_pool(name="ps", bufs=3, space="PSUM") as psp, \
             tc.tile_pool(name="psf", bufs=2, space="PSUM") as psfp, \
             tc.tile_pool(name="psy", bufs=2, space="PSUM") as psyp:

            def load_const(name, shape, dt=F32):
                t = cpool.tile(list(shape), dt, tag=name, name=f"c_{name}")
                nc.sync.dma_start(t[:], inp[name][tuple(slice(None) for _ in shape)])
                return t

            ident_t = load_const("ident", [128, 128])
            W = {}
            for li in (1, 2):
                for nm, shp in (("wl_comb", [128, 128]), ("wr_in", [128, 64]),
                                ("wr_out", [128, 64]), ("bias_pk", [128, 1]),
                                ("wcx", [128, 128]), ("wch", [128, 128]),
                                ("cb", [128, 1])):
                    W[f"{nm}{li}"] = load_const(f"{nm}{li}", shp)
            fw_t = load_const("fw", [128, 1])
            fb_t = load_const("fb", [1, 1])
            recip_t = {d: load_const(f"recip_{d}", [128, NCH]) for d in dims}
            unperm_t = {}
            for d in dims:
                for h in ("A", "B"):
                    ut = idxp.tile([128, NLOCP // 16], I16, tag=f"up_{d}_{h}", name=f"up_{d}_{h}")
                    nc.sync.dma_start(ut[:], inp[f"unperm_{d}_{h}"][:, :])
                    unperm_t[d, h] = ut

            zero_t = smallp.tile([128, 128], F32, tag="zero")
            nc.vector.memset(zero_t[:], 0.0)

            def zero_rows(tab, start):
                nc.sync.dma_start(
                    tab[start:start + 128, :].rearrange("(k p) c -> p k c", p=128),
                    zero_t[:].rearrange("p (k c) -> p k c", k=1))

            zero_rows(y1tab, 0)
            zero_rows(y1tab, TROWS - 128)
            zero_rows(y2tab, 0)
            zero_rows(y2tab, ZHEAD + N)          # rows 50128..50256
            zero_rows(y2tab, TROWS - 128)        # rows 50176..50304

            # ---------------- y1 table build (replicated over full graph)
            t_off = 0
            while t_off < NFULLP:
                tw = min(512, NFULLP - t_off)
                nchk = tw // 128
                xs = ybldp.tile([128, 512], F32, tag="xs")
                nc.sync.dma_start(xs[:, 0:tw], xt_full[:, t_off:t_off + tw])
                ps = psyp.tile([128, 512], F32, tag="psy")
                for k in range(nchk):
                    nc.tensor.matmul(ps[:, 128 * k:128 * (k + 1)],
                                     xs[:, 128 * k:128 * (k + 1)],
                                     W["wl_comb1"][:], start=True, stop=True)
                ys = ystgp.tile([128, 512], F32, tag="ys")
                nc.scalar.activation(ys[:, 0:tw], ps[:, 0:tw], COPY)
                nc.sync.dma_start(
                    y1tab[ZHEAD + t_off:ZHEAD + t_off + tw, :]
                    .rearrange("(k p) c -> p k c", p=128),
                    ys[:, 0:tw].rearrange("p (k c) -> p k c", k=nchk))
                t_off += tw

            # ---------------- shared helpers
            def aggregate(d, table, mean_tiles):
                """Gather+accumulate both halves of direction d from table,
                unpermute, merge, scale; result into mean_tiles[d]."""
                dinfo = plan["dirs"][d]
                col0 = 0 if d == "in" else 64
                accs = {}
                for h in ("A", "B"):
                    hinfo = dinfo["halves"][h]
                    if hinfo["is_a"]:
                        tab_ap = table[0:BOFF, col0:col0 + 64]
                    else:
                        tab_ap = table[BOFF:TROWS, col0:col0 + 64]
                    acc = accp.tile([128, NCH, 64], F32, tag="acc")
                    for goff, rounds in hinfo["groups"]:
                        rows = sum(nr for _, nr, _ in rounds)
                        gidx = idxgp.tile([128, SMAX // 16], I16, tag="gidx")
                        nc.sync.dma_start(
                            gidx[:, 0:rows // 16],
                            inp[f"stream_{d}_{h}"][:, goff // 16:(goff + rows) // 16])
                        stg = stgp.tile([128, SMAX // 128, 64], F32, tag="stg")
                        nc.gpsimd.dma_gather(
                            stg[:, 0:rows // 128, :], tab_ap,
                            gidx[:, 0:rows // 16],
                            num_idxs=rows, num_idxs_reg=rows,
                            elem_size=64, elem_step=128, single_packet=False)
                        if DEBUG and d == "in" and table is y1tab and h == "B":
                            gi = hinfo["groups"].index((goff, rounds))
                            nc.sync.dma_start(
                                dbg["dbg_stgB"][gi * SMAX:gi * SMAX + rows, :]
                                .rearrange("(c p) f -> p c f", p=128),
                                stg[:, 0:rows // 128, :])
                        for loff, nr, r in rounds:
                            cr = nr // 128
                            s_ap = stg[:, loff // 128:loff // 128 + cr, :]
                            if r == 0:
                                nc.vector.tensor_copy(acc[:], s_ap)
                            else:
                                nc.vector.tensor_add(
                                    acc[:, 0:cr, :], acc[:, 0:cr, :], s_ap)
                    accs[h] = acc
                hidx = {"A": 0, "B": 1}
                for h in ("A", "B"):
                    nc.sync.dma_start(
                        accd[d][hidx[h], :, :].rearrange("(c p) f -> p c f", p=128),
                        accs[h][:])
                    if DEBUG and d == "in" and table is y1tab:
                        nc.sync.dma_start(
                            dbg[f"dbg_acc{h}"][:, :].rearrange("(c p) f -> p c f", p=128),
                            accs[h][:])
                nats = {}
                for h in ("A", "B"):
                    nat = stgp.tile([128, NCH, 64], F32, tag="stg")
                    nc.gpsimd.dma_gather(
                        nat[:], accd[d][hidx[h], :, :], unperm_t[d, h][:],
                        num_idxs=NLOCP, num_idxs_reg=NLOCP,
                        elem_size=64, elem_step=64, single_packet=False)
                    nats[h] = nat
                    if DEBUG and d == "in" and table is y1tab:
                        nc.sync.dma_start(
                            dbg[f"dbg_nat{h}"][:, :].rearrange("(c p) f -> p c f", p=128),
                            nat[:])
                mean = mean_tiles[d]
                nc.vector.tensor_add(mean[:], nats["A"][:], nats["B"][:])
                rb = recip_t[d][:].unsqueeze(2).broadcast_to((128, NCH, 64))
                nc.vector.tensor_mul(mean[:], mean[:], rb)

            def seg_widths():
                segs = []
                off = 0
                while off < NLOCP:
                    w = min(512, NLOCP - off)
                    segs.append((off, w))
                    off += w
                return segs

            def h_pass(li, get_feat, mean_tiles, h_t):
                """h_packed = relu(meanT + Wr.T @ featT + bias)."""
                for off, w in seg_widths():
                    feat_ap = get_feat(off, w)
                    ps = psp.tile([128, 512], F32, tag="ps")
                    nch = w // 128
                    nc.tensor.matmul(ps[0:64, 0:w], W[f"wr_in{li}"][:],
                                     feat_ap, start=True, stop=False)
                    nc.tensor.matmul(ps[64:128, 0:w], W[f"wr_out{li}"][:],
                                     feat_ap, start=True, stop=False,
                                     tile_position=(0, 64))
                    for k in range(nch):
                        c = (off + 128 * k) // 128
                        last = k == nch - 1
                        nc.tensor.matmul(ps[0:64, 128 * k:128 * (k + 1)],
                                         mean_tiles["in"][:, c, :], ident_t[:],
                                         start=False, stop=last)
                        nc.tensor.matmul(ps[64:128, 128 * k:128 * (k + 1)],
                                         mean_tiles["out"][:, c, :], ident_t[:],
                                         start=False, stop=last,
                                         tile_position=(0, 64))
                    nc.scalar.activation(h_t[:, off:off + w], ps[:, 0:w], RELU,
                                         bias=W[f"bias_pk{li}"][:])

            def comb_pass(li, get_feat, h_t, out_cb):
                for off, w in seg_widths():
                    ps = psp.tile([128, 512], F32, tag="ps")
                    nc.tensor.matmul(ps[:, 0:w], W[f"wcx{li}"][:],
                                     get_feat(off, w), start=True, stop=False)
                    nc.tensor.matmul(ps[:, 0:w], W[f"wch{li}"][:],
                                     h_t[:, off:off + w], start=False, stop=True)
                    out_cb(off, w, ps)

            def xt_seg(off, w):
                xs = ybldp.tile([128, 512], F32, tag="xseg")
                nc.sync.dma_start(xs[:, 0:w], xt_loc[:, off:off + w])
                return xs[:, 0:w]

            # ---------------- layer 1
            mean_tiles = {d: meanp.tile([128, NCH, 64], F32, tag="mean",
                                        name=f"mean1_{d}")
                          for d in dims}
            for d in dims:
                aggregate(d, y1tab, mean_tiles)
            if DEBUG:
                for d in dims:
                    nc.sync.dma_start(
                        dbg[f"dbg_mean_{d}1"][:, :].rearrange("(c p) f -> p c f", p=128),
                        mean_tiles[d][:])
            h1_t = featp.tile([128, NLOCP], F32, tag="bigfeat")
            h_pass(1, xt_seg, mean_tiles, h1_t)
            if DEBUG:
                nc.sync.dma_start(dbg["dbg_h1"][:, :], h1_t[:])
            x2_t = featp.tile([128, NLOCP], F32, tag="bigfeat")

            def l1_out(off, w, ps):
                nc.scalar.activation(x2_t[:, off:off + w], ps[:, 0:w], RELU,
                                     bias=W["cb1"][:])
            comb_pass(1, xt_seg, h1_t, l1_out)
            if DEBUG:
                nc.sync.dma_start(dbg["dbg_x2"][:, :], x2_t[:])

            # y2 table slice + AllGather
            for g in range((NCH + 3) // 4):
                c0 = 4 * g
                ncc = min(4, NCH - c0)
                ps = psyp.tile([128, 512], F32, tag="psy")
                for k in range(ncc):
                    nc.tensor.matmul(ps[:, 128 * k:128 * (k + 1)],
                                     x2_t[:, 128 * (c0 + k):128 * (c0 + k + 1)],
                                     W["wl_comb2"][:], start=True, stop=True)
                ys = ystgp.tile([128, 512], F32, tag="ys")
                nc.scalar.activation(ys[:, 0:128 * ncc], ps[:, 0:128 * ncc], COPY)
                nc.sync.dma_start(
                    y2slice[128 * c0:128 * (c0 + ncc), :]
                    .rearrange("(k p) c -> p k c", p=128),
                    ys[:, 0:128 * ncc].rearrange("p (k c) -> p k c", k=ncc))
            nc.gpsimd.collective_compute(
                "AllGather", mybir.AluOpType.bypass,
                replica_groups=[list(range(NC))],
                ins=[y2slice[0:NLOC, :]],
                outs=[y2tab[ZHEAD:ZHEAD + N, :]],
            )

            # ---------------- layer 2
            mean_tiles2 = {d: meanp.tile([128, NCH, 64], F32, tag="mean",
                                         name=f"mean2_{d}")
                           for d in dims}
            for d in dims:
                aggregate(d, y2tab, mean_tiles2)
            h2_t = featp.tile([128, NLOCP], F32, tag="bigfeat")

            def x2_seg(off, w):
                return x2_t[:, off:off + w]
            h_pass(2, x2_seg, mean_tiles2, h2_t)

            outsb = smallp.tile([1, NLOCP], F32, tag="outsb")

            def l2_out(off, w, ps):
                x3 = ystgp.tile([128, 512], F32, tag="x3")
                nc.scalar.activation(x3[:, 0:w], ps[:, 0:w], RELU,
                                     bias=W["cb2"][:])
                psf = psfp.tile([1, 512], F32, tag="psf")
                nc.tensor.matmul(psf[0:1, 0:w], fw_t[:], x3[:, 0:w],
                                 start=True, stop=True)
                nc.vector.tensor_scalar_add(outsb[0:1, off:off + w],
                                            psf[0:1, 0:w], fb_t[0:1, 0:1])
            comb_pass(2, x2_seg, h2_t, l2_out)

            nc.sync.dma_start(out_t[0:1, :], outsb[0:1, 0:NLOC])

    nc.compile()
    return nc


# ------------------------------------------------------------------ interface

def _make_in_maps(plan, inputs):
    x = np.asarray(inputs["x"], dtype=np.float32)
    xt = np.zeros((128, NFULLP), dtype=np.float32)
    xt[:, :N] = np.ascontiguousarray(x.T)
    ident = np.eye(128, dtype=np.float32)

    def cat(a, b):
        return np.ascontiguousarray(
            np.concatenate([np.asarray(a, np.float32), np.asarray(b, np.float32)],
                           axis=1))

    common = {
        "xt_full": xt,
        "ident": ident,
        "wl_comb1": cat(inputs["in_Wl0"], inputs["out_Wl0"]),
        "wr_in1": np.asarray(inputs["in_Wr0"], np.float32),
        "wr_out1": np.asarray(inputs["out_Wr0"], np.float32),
        "bias_pk1": np.concatenate(
            [np.asarray(inputs["in_bl0"], np.float32),
             np.asarray(inputs["out_bl0"], np.float32)])[:, None].copy(),
        "wcx1": np.ascontiguousarray(np.asarray(inputs["comb_W0"], np.float32)[0:128]),
        "wch1": np.ascontiguousarray(np.asarray(inputs["comb_W0"], np.float32)[128:256]),
        "cb1": np.asarray(inputs["comb_b0"], np.float32)[:, None].copy(),
        "wl_comb2": cat(inputs["in_Wl1"], inputs["out_Wl1"]),
        "wr_in2": np.asarray(inputs["in_Wr1"], np.float32),
        "wr_out2": np.asarray(inputs["out_Wr1"], np.float32),
        "bias_pk2": np.concatenate(
            [np.asarray(inputs["in_bl1"], np.float32),
             np.asarray(inputs["out_bl1"], np.float32)])[:, None].copy(),
        "wcx2": np.ascontiguousarray(np.asarray(inputs["comb_W1"], np.float32)[0:128]),
        "wch2": np.ascontiguousarray(np.asarray(inputs["comb_W1"], np.float32)[128:256]),
        "cb2": np.asarray(inputs["comb_b1"], np.float32)[:, None].copy(),
        "fw": np.asarray(inputs["final_W"], np.float32).reshape(128, 1).copy(),
        "fb": np.asarray(inputs["final_b"], np.float32).reshape(1, 1).copy(),
    }
    in_maps = []
    for c in range(NC):
        m = dict(common)
        xl = np.zeros((128, NLOCP), dtype=np.float32)
        xl[:, :NLOC] = x.T[:, c * NLOC:(c + 1) * NLOC]
        m["xt_loc"] = xl
        for d in ("in", "out"):
            dinfo = plan["dirs"][d]
            rc = np.zeros((128, NCH), dtype=np.float32)
            r = dinfo["recip"][c]  # [NLOCP]
            rc[:, :] = r.reshape(NCH, 128).T
            m[f"recip_{d}"] = rc.copy()
            for h in ("A", "B"):
                hinfo = dinfo["halves"][h]
                m[f"stream_{d}_{h}"] = _wrap_idx(hinfo["streams"][c])
                m[f"unperm_{d}_{h}"] = _wrap_idx(hinfo["unperm"][c])
        in_maps.append(m)
    return in_maps


def kernel(**inputs):
    plan = _preprocess(np.asarray(inputs["edge_index_in"]),
                       np.asarray(inputs["edge_index_out"]))
    key = tuple(
        (d, h, tuple(plan["dirs"][d]["halves"][h]["NR"]))
        for d in ("in", "out") for h in ("A", "B"))
    if key not in _CACHE:
        _CACHE[key] = _build_program(plan)
    nc = _CACHE[key]
    in_maps = _make_in_maps(plan, inputs)
    res = bass_utils.run_bass_kernel_spmd(nc, in_maps, core_ids=list(range(NC)))
    out = np.concatenate([r["out"][0] for r in res.results])
    return out.astype(np.float32)


# revision 21
# speedup vs baseline: 1.8787x; 1.8787x over previous
"""DirectedDualSAGE (2-layer dual-direction GraphSAGE + MLP head) on 8 trn2
NeuronCores via Bass/Tile.

Strategy (dst-sharded):
- Nodes block-sharded 6250/core; each core owns all edges whose dst falls in
  its shard (both directions).
- Per layer, the SAGE "lin_l(mean(x_j))" term is computed as
  diag(1/cnt) * A * (x @ Wl): transform first (64-dim), then aggregate, which
  halves gather traffic. Aggregation = per-edge row gather (dma_gather) of the
  transformed feature table + segmented accumulation on the vector engine
  using degree-sorted "rounds" (round r adds the r-th edge of every dst; after
  sorting dsts by descending degree each round is a prefix, so the add is one
  contiguous tensor_tensor op).
- Edge srcs span all nodes but dma_gather indices are int16, so the feature
  table is split in two halves (src < 25088 / >= 25088) with per-half streams,
  accumulators and zero-pad rows.
- Layer 1's table (x @ [Wl_in|Wl_out]) is computed replicated on every core
  (x is an input, so no communication). Layer 2's table is computed from the
  local x2 shard and AllGather'ed.
- All dense math runs feature-major on the tensor engine; the aggregated means
  (node-major from the gather) are transposed back via PE identity-matmuls that
  accumulate into the same PSUM as the x @ Wr term.

kernel(**inputs) takes the full unsharded inputs and returns the full [N]
output; sharding happens inside.
"""

import numpy as np

import concourse.bacc as bacc
import concourse.tile as tile
import concourse.mybir as mybir
from concourse import bass_utils

F32 = mybir.dt.float32
I16 = mybir.dt.int16

N = 50000
NC = 8
NLOC = N // NC            # 6250
NLOCP = 6272              # 49*128
NCH = NLOCP // 128        # 49 chunks
NFULLP = 50048            # 391*128
HALF = 25088              # src half split
ZHEAD = 128               # zero rows at table head
TROWS = ZHEAD + NFULLP + 128   # 50304
BOFF = ZHEAD + HALF       # 25216, table-B base row
BROWS = TROWS - BOFF      # 25088
BZERO = ZHEAD + NFULLP - BOFF  # 24960 -> row 50176 (zero tail)
SMAX = 3200               # max rows per dma_gather call
NQ = 4                    # SWDGE queues

_CACHE = {}
DEBUG = False


# ----------------------------------------------------------------- host prep

def _round_up(v, m):
    return (v + m - 1) // m * m


def _per_core_half(src, dst, half_mask):
    out = []
    for c in range(NC):
        m = (dst // NLOC == c) & half_mask
        s = src[m]
        dloc = (dst[m] - c * NLOC).astype(np.int64)
        deg = np.bincount(dloc, minlength=NLOCP).astype(np.int64)
        perm = np.argsort(-deg, kind="stable").astype(np.int64)
        pos = np.empty(NLOCP, dtype=np.int64)
        pos[perm] = np.arange(NLOCP)
        order = np.argsort(dloc, kind="stable")
        sd = dloc[order]
        ss = s[order]
        if len(sd):
            starts = np.r_[0, 1 + np.flatnonzero(np.diff(sd))]
            group_id = np.zeros(len(sd), dtype=np.int64)
            group_id[starts[1:]] = 1
            group_id = np.cumsum(group_id)
            rank = np.arange(len(sd)) - starts[group_id]
        else:
            rank = sd
        slot = pos[sd]
        maxdeg = int(deg.max()) if len(sd) else 0
        rounds = []
        for r in range(maxdeg):
            mr = rank == r
            rounds.append((int(np.count_nonzero(mr)), slot[mr], ss[mr]))
        out.append(dict(deg=deg, pos=pos, rounds=rounds))
    return out


def _preprocess(edge_index_in, edge_index_out):
    plan = {"dirs": {}}
    for dname, ei in (("in", edge_index_in), ("out", edge_index_out)):
        src = ei[0].astype(np.int64)
        dst = ei[1].astype(np.int64)
        dinfo = {"halves": {}, "recip": []}
        for c in range(NC):
            m = dst // NLOC == c
            dloc = dst[m] - c * NLOC
            cnt = np.bincount(dloc, minlength=NLOCP).astype(np.float32)
            dinfo["recip"].append((1.0 / np.maximum(cnt, 1.0)).astype(np.float32))
        for hname, is_a in (("A", True), ("B", False)):
            half_mask = (src < HALF) if is_a else (src >= HALF)
            cores = _per_core_half(src, dst, half_mask)
            nrounds = max(len(ci["rounds"]) for ci in cores)
            NR = []
            for r in range(nrounds):
                mx = max((ci["rounds"][r][0] if r < len(ci["rounds"]) else 0)
                         for ci in cores)
                NR.append(_round_up(max(mx, 1), 128))
            NR[0] = NLOCP  # full first round: copy-initializes the accumulator
            zi = 0 if is_a else BZERO
            streams = []
            for ci in cores:
                parts = []
                for r in range(nrounds):
                    vec = np.full(NR[r], zi, dtype=np.int64)
                    if r < len(ci["rounds"]):
                        _, slots, ss = ci["rounds"][r]
                        vec[slots] = (ss + ZHEAD) if is_a else (ss - HALF)
                    parts.append(vec)
                stream = np.concatenate(parts)
                assert stream.max(initial=0) < 32768
                streams.append(stream.astype(np.int16))
            # uniform SMAX-row cuts; rounds may split across groups (the
            # per-fragment add targets acc slot range [s0, s1))
            L = int(sum(NR))
            groups = []  # (stream_off, [(stg_off, acc_slot_off, nrows, r)])
            r, r_off = 0, 0
            off = 0
            while off < L:
                rows = min(SMAX, L - off)
                frags = []
                done = 0
                while done < rows:
                    take = min(NR[r] - r_off, rows - done)
                    frags.append((done, r_off, take, r))
                    done += take
                    r_off += take
                    if r_off == NR[r]:
                        r += 1
                        r_off = 0
                groups.append((off, frags))
                off += rows
            dinfo["halves"][hname] = dict(
                NR=NR, L=int(sum(NR)), streams=streams, groups=groups,
                unperm=[ci["pos"].astype(np.int16) for ci in cores], is_a=is_a,
            )
        plan["dirs"][dname] = dinfo
    return plan


def _wrap_idx(idx):
    L = idx.shape[0]
    assert L % 16 == 0
    w = idx.reshape(L // 16, 16).T.astype(np.int16)
    return np.ascontiguousarray(np.tile(w, (8, 1)))


# ------------------------------------------------------------- device program

def _build_program(plan):
    nc = bacc.Bacc("TRN2", target_bir_lowering=False, debug=False,
                   num_devices=NC, num_swdge_queues=NQ)
    dims = ("in", "out")
    inp = {}

    def dram_in(name, shape, dt=F32):
        inp[name] = nc.dram_tensor(name, list(shape), dt, kind="ExternalInput")
        return inp[name]

    xt_full = dram_in("xt_full", [128, NFULLP])
    xt_loc = dram_in("xt_loc", [128, NLOCP])
    ident = dram_in("ident", [128, 128])
    for li in (1, 2):
        dram_in(f"wl_comb{li}", [128, 128])
        dram_in(f"wr_in{li}", [128, 64])
        dram_in(f"wr_out{li}", [128, 64])
        dram_in(f"bias_pk{li}", [128, 1])
        dram_in(f"wcx{li}", [128, 128])
        dram_in(f"wch{li}", [128, 128])
        dram_in(f"cb{li}", [128, 1])
    dram_in("fw", [128, 1])
    dram_in("fb", [1, 1])
    for d in dims:
        dram_in(f"recip_{d}", [128, NCH])
        for h in ("A", "B"):
            L = plan["dirs"][d]["halves"][h]["L"]
            dram_in(f"stream_{d}_{h}", [128, L // 16], I16)
            dram_in(f"unperm_{d}_{h}", [128, NLOCP // 16], I16)

    y1tab = nc.dram_tensor("y1tab", [TROWS, 128], F32, kind="Internal")
    y2tab = nc.dram_tensor("y2tab", [TROWS, 128], F32, kind="Internal",
                           addr_space="Shared")
    y2slice = nc.dram_tensor("y2slice", [NLOCP, 128], F32, kind="Internal")
    accd = {d: nc.dram_tensor(f"accd_{d}", [2, NLOCP, 64], F32, kind="Internal")
            for d in dims}
    out_t = nc.dram_tensor("out", [1, NLOC], F32, kind="ExternalOutput")
    dbg = {}
    if DEBUG:
        for nm, shp in (("dbg_mean_in1", [NLOCP, 64]), ("dbg_mean_out1", [NLOCP, 64]),
                        ("dbg_h1", [128, NLOCP]), ("dbg_x2", [128, NLOCP]),
                        ("dbg_accA", [NLOCP, 64]), ("dbg_accB", [NLOCP, 64]),
                        ("dbg_natA", [NLOCP, 64]), ("dbg_natB", [NLOCP, 64])):
            dbg[nm] = nc.dram_tensor(nm, shp, F32, kind="ExternalOutput")

    _qctr = [0]

    def next_queue():
        q = _qctr[0] % NQ
        _qctr[0] += 1
        return q

    RELU = mybir.ActivationFunctionType.Relu
    COPY = mybir.ActivationFunctionType.Copy

    with tile.TileContext(nc) as tc:
        with tc.tile_pool(name="const", bufs=1) as cpool, \
             tc.tile_pool(name="idxp", bufs=1) as idxp, \
             tc.tile_pool(name="idxg", bufs=8) as idxgp, \
             tc.tile_pool(name="feat", bufs=2) as featp, \
             tc.tile_pool(name="accp", bufs=2) as accp, \
             tc.tile_pool(name="meanp", bufs=2) as meanp, \
             tc.tile_pool(name="stg", bufs=6) as stgp, \
             tc.tile_pool(name="natp", bufs=2) as natp, \
             tc.tile_pool(name="ybld", bufs=2) as ybldp, \
             tc.tile_pool(name="ystg", bufs=2) as ystgp, \
             tc.tile_pool(name="small", bufs=1) as smallp, \
             tc.tile_pool(name="ps", bufs=3, space="PSUM") as psp, \
             tc.tile_pool(name="psf", bufs=2, space="PSUM") as psfp, \
             tc.tile_pool(name="psy", bufs=2, space="PSUM") as psyp:

            def load_const(name, shape, dt=F32):
                t = cpool.tile(list(shape), dt, tag=name, name=f"c_{name}")
                nc.sync.dma_start(t[:], inp[name][tuple(slice(None) for _ in shape)])
                return t

            ident_t = load_const("ident", [128, 128])
            W = {}
            for li in (1, 2):
                for nm, shp in (("wl_comb", [128, 128]), ("wr_in", [128, 64]),
                                ("wr_out", [128, 64]), ("bias_pk", [128, 1]),
                                ("wcx", [128, 128]), ("wch", [128, 128]),
                                ("cb", [128, 1])):
                    W[f"{nm}{li}"] = load_const(f"{nm}{li}", shp)
            fw_t = load_const("fw", [128, 1])
            fb_t = load_const("fb", [1, 1])
            recip_t = {d: load_const(f"recip_{d}", [128, NCH]) for d in dims}
            unperm_t = {}
            for d in dims:
                for h in ("A", "B"):
                    ut = idxp.tile([128, NLOCP // 16], I16, tag=f"up_{d}_{h}", name=f"up_{d}_{h}")
                    nc.sync.dma_start(ut[:], inp[f"unperm_{d}_{h}"][:, :])
                    unperm_t[d, h] = ut

            zero_t = smallp.tile([128, 128], F32, tag="zero")
            nc.vector.memset(zero_t[:], 0.0)

            def zero_rows(tab, start):
                nc.sync.dma_start(
                    tab[start:start + 128, :].rearrange("(k p) c -> p k c", p=128),
                    zero_t[:].rearrange("p (k c) -> p k c", k=1))

            zero_rows(y1tab, 0)
            zero_rows(y1tab, TROWS - 128)
            zero_rows(y2tab, 0)
            zero_rows(y2tab, ZHEAD + N)          # rows 50128..50256
            zero_rows(y2tab, TROWS - 128)        # rows 50176..50304

            # ---------------- y1 table build (replicated over full graph)
            t_off = 0
            while t_off < NFULLP:
                tw = min(512, NFULLP - t_off)
                nchk = tw // 128
                xs = ybldp.tile([128, 512], F32, tag="xs")
                nc.sync.dma_start(xs[:, 0:tw], xt_full[:, t_off:t_off + tw])
                ps = psyp.tile([128, 512], F32, tag="psy")
                for k in range(nchk):
                    nc.tensor.matmul(ps[:, 128 * k:128 * (k + 1)],
                                     xs[:, 128 * k:128 * (k + 1)],
                                     W["wl_comb1"][:], start=True, stop=True)
                ys = ystgp.tile([128, 512], F32, tag="ys")
                nc.scalar.activation(ys[:, 0:tw], ps[:, 0:tw], COPY)
                nc.sync.dma_start(
                    y1tab[ZHEAD + t_off:ZHEAD + t_off + tw, :]
                    .rearrange("(k p) c -> p k c", p=128),
                    ys[:, 0:tw].rearrange("p (k c) -> p k c", k=nchk))
                t_off += tw

            # ---------------- shared helpers
            def aggregate(d, table, mean_tiles):
                """Gather+accumulate both halves of direction d from table,
                unpermute, merge, scale; result into mean_tiles[d]."""
                dinfo = plan["dirs"][d]
                col0 = 0 if d == "in" else 64
                accs = {}
                for h in ("A", "B"):
                    hinfo = dinfo["halves"][h]
                    if hinfo["is_a"]:
                        tab_ap = table[0:BOFF, col0:col0 + 64]
                    else:
                        tab_ap = table[BOFF:TROWS, col0:col0 + 64]
                    acc = accp.tile([128, NCH, 64], F32, tag="acc")
                    for goff, frags in hinfo["groups"]:
                        rows = sum(f[2] for f in frags)
                        gidx = idxgp.tile([128, SMAX // 16], I16, tag="gidx")
                        nc.sync.dma_start(
                            gidx[:, 0:rows // 16],
                            inp[f"stream_{d}_{h}"][:, goff // 16:(goff + rows) // 16])
                        stg = stgp.tile([128, SMAX // 128, 64], F32, tag="stg")
                        nc.gpsimd.dma_gather(
                            stg[:, 0:rows // 128, :], tab_ap,
                            gidx[:, 0:rows // 16],
                            num_idxs=rows, num_idxs_reg=rows,
                            elem_size=64, elem_step=128, single_packet=False,
                            queue_num=next_queue())
                        for stg_off, slot_off, nrows, r in frags:
                            cr = nrows // 128
                            c0 = slot_off // 128
                            s_ap = stg[:, stg_off // 128:stg_off // 128 + cr, :]
                            a_ap = acc[:, c0:c0 + cr, :]
                            if r == 0:
                                nc.vector.tensor_copy(a_ap, s_ap)
                            else:
                                nc.vector.tensor_add(a_ap, a_ap, s_ap)
                    accs[h] = acc
                hidx = {"A": 0, "B": 1}
                for h in ("A", "B"):
                    nc.sync.dma_start(
                        accd[d][hidx[h], :, :].rearrange("(c p) f -> p c f", p=128),
                        accs[h][:])
                    if DEBUG and d == "in" and table is y1tab:
                        nc.sync.dma_start(
                            dbg[f"dbg_acc{h}"][:, :].rearrange("(c p) f -> p c f", p=128),
                            accs[h][:])
                nats = {}
                for h in ("A", "B"):
                    nat = natp.tile([128, NCH, 64], F32, tag="nat")
                    nc.gpsimd.dma_gather(
                        nat[:], accd[d][hidx[h], :, :], unperm_t[d, h][:],
                        num_idxs=NLOCP, num_idxs_reg=NLOCP,
                        elem_size=64, elem_step=64, single_packet=False,
                        queue_num=next_queue())
                    nats[h] = nat
                    if DEBUG and d == "in" and table is y1tab:
                        nc.sync.dma_start(
                            dbg[f"dbg_nat{h}"][:, :].rearrange("(c p) f -> p c f", p=128),
                            nat[:])
                mean = mean_tiles[d]
                nc.vector.tensor_add(mean[:], nats["A"][:], nats["B"][:])
                rb = recip_t[d][:].unsqueeze(2).broadcast_to((128, NCH, 64))
                nc.vector.tensor_mul(mean[:], mean[:], rb)

            def seg_widths():
                segs = []
                off = 0
                while off < NLOCP:
                    w = min(512, NLOCP - off)
                    segs.append((off, w))
                    off += w
                return segs

            def h_pass(li, get_feat, mean_tiles, h_t):
                """h_packed = relu(meanT + Wr.T @ featT + bias)."""
                for off, w in seg_widths():
                    feat_ap = get_feat(off, w)
                    ps = psp.tile([128, 512], F32, tag="ps")
                    nch = w // 128
                    nc.tensor.matmul(ps[0:64, 0:w], W[f"wr_in{li}"][:],
                                     feat_ap, start=True, stop=False)
                    nc.tensor.matmul(ps[64:128, 0:w], W[f"wr_out{li}"][:],
                                     feat_ap, start=True, stop=False,
                                     tile_position=(0, 64))
                    for k in range(nch):
                        c = (off + 128 * k) // 128
                        last = k == nch - 1
                        nc.tensor.matmul(ps[0:64, 128 * k:128 * (k + 1)],
                                         mean_tiles["in"][:, c, :], ident_t[:],
                                         start=False, stop=last)
                        nc.tensor.matmul(ps[64:128, 128 * k:128 * (k + 1)],
                                         mean_tiles["out"][:, c, :], ident_t[:],
                                         start=False, stop=last,
                                         tile_position=(0, 64))
                    nc.scalar.activation(h_t[:, off:off + w], ps[:, 0:w], RELU,
                                         bias=W[f"bias_pk{li}"][:])

            def comb_pass(li, get_feat, h_t, out_cb):
                for off, w in seg_widths():
                    ps = psp.tile([128, 512], F32, tag="ps")
                    nc.tensor.matmul(ps[:, 0:w], W[f"wcx{li}"][:],
                                     get_feat(off, w), start=True, stop=False)
                    nc.tensor.matmul(ps[:, 0:w], W[f"wch{li}"][:],
                                     h_t[:, off:off + w], start=False, stop=True)
                    out_cb(off, w, ps)

            def xt_seg(off, w):
                xs = ybldp.tile([128, 512], F32, tag="xseg")
                nc.sync.dma_start(xs[:, 0:w], xt_loc[:, off:off + w])
                return xs[:, 0:w]

            # ---------------- layer 1
            mean_tiles = {d: meanp.tile([128, NCH, 64], F32, tag="mean",
                                        name=f"mean1_{d}")
                          for d in dims}
            for d in dims:
                aggregate(d, y1tab, mean_tiles)
            if DEBUG:
                for d in dims:
                    nc.sync.dma_start(
                        dbg[f"dbg_mean_{d}1"][:, :].rearrange("(c p) f -> p c f", p=128),
                        mean_tiles[d][:])
            h1_t = featp.tile([128, NLOCP], F32, tag="bigfeat")
            h_pass(1, xt_seg, mean_tiles, h1_t)
            if DEBUG:
                nc.sync.dma_start(dbg["dbg_h1"][:, :], h1_t[:])
            x2_t = featp.tile([128, NLOCP], F32, tag="bigfeat")

            def l1_out(off, w, ps):
                nc.scalar.activation(x2_t[:, off:off + w], ps[:, 0:w], RELU,
                                     bias=W["cb1"][:])
            comb_pass(1, xt_seg, h1_t, l1_out)
            if DEBUG:
                nc.sync.dma_start(dbg["dbg_x2"][:, :], x2_t[:])

            # y2 table slice + AllGather
            for g in range((NCH + 3) // 4):
                c0 = 4 * g
                ncc = min(4, NCH - c0)
                ps = psyp.tile([128, 512], F32, tag="psy")
                for k in range(ncc):
                    nc.tensor.matmul(ps[:, 128 * k:128 * (k + 1)],
                                     x2_t[:, 128 * (c0 + k):128 * (c0 + k + 1)],
                                     W["wl_comb2"][:], start=True, stop=True)
                ys = ystgp.tile([128, 512], F32, tag="ys")
                nc.scalar.activation(ys[:, 0:128 * ncc], ps[:, 0:128 * ncc], COPY)
                nc.sync.dma_start(
                    y2slice[128 * c0:128 * (c0 + ncc), :]
                    .rearrange("(k p) c -> p k c", p=128),
                    ys[:, 0:128 * ncc].rearrange("p (k c) -> p k c", k=ncc))
            nc.gpsimd.collective_compute(
                "AllGather", mybir.AluOpType.bypass,
                replica_groups=[list(range(NC))],
                ins=[y2slice[0:NLOC, :]],
                outs=[y2tab[ZHEAD:ZHEAD + N, :]],
            )

            # ---------------- layer 2
            mean_tiles2 = {d: meanp.tile([128, NCH, 64], F32, tag="mean",
                                         name=f"mean2_{d}")
                           for d in dims}
            for d in dims:
                aggregate(d, y2tab, mean_tiles2)
            h2_t = featp.tile([128, NLOCP], F32, tag="bigfeat")

            def x2_seg(off, w):
                return x2_t[:, off:off + w]
            h_pass(2, x2_seg, mean_tiles2, h2_t)

            def l2_out(off, w, ps):
                x3 = ystgp.tile([128, 512], F32, tag="x3")
                nc.scalar.activation(x3[:, 0:w], ps[:, 0:w], RELU,
                                     bias=W["cb2"][:])
                psf = psfp.tile([1, 512], F32, tag="psf")
                nc.tensor.matmul(psf[0:1, 0:w], fw_t[:], x3[:, 0:w],
                                 start=True, stop=True)
                osb = ystgp.tile([1, 512], F32, tag="osb")
                nc.vector.tensor_scalar_add(osb[0:1, 0:w],
                                            psf[0:1, 0:w], fb_t[0:1, 0:1])
                wv = min(w, NLOC - off)
                if wv > 0:
                    nc.sync.dma_start(out_t[0:1, off:off + wv], osb[0:1, 0:wv])
            comb_pass(2, x2_seg, h2_t, l2_out)

    nc.compile()
    return nc


# ------------------------------------------------------------------ interface

def _make_in_maps(plan, inputs):
    x = np.asarray(inputs["x"], dtype=np.float32)
    xt = np.zeros((128, NFULLP), dtype=np.float32)
    xt[:, :N] = np.ascontiguousarray(x.T)
    ident = np.eye(128, dtype=np.float32)

    def cat(a, b):
        return np.ascontiguousarray(
            np.concatenate([np.asarray(a, np.float32), np.asarray(b, np.float32)],
                           axis=1))

    common = {
        "xt_full": xt,
        "ident": ident,
        "wl_comb1": cat(inputs["in_Wl0"], inputs["out_Wl0"]),
        "wr_in1": np.asarray(inputs["in_Wr0"], np.float32),
        "wr_out1": np.asarray(inputs["out_Wr0"], np.float32),
        "bias_pk1": np.concatenate(
            [np.asarray(inputs["in_bl0"], np.float32),
             np.asarray(inputs["out_bl0"], np.float32)])[:, None].copy(),
        "wcx1": np.ascontiguousarray(np.asarray(inputs["comb_W0"], np.float32)[0:128]),
        "wch1": np.ascontiguousarray(np.asarray(inputs["comb_W0"], np.float32)[128:256]),
        "cb1": np.asarray(inputs["comb_b0"], np.float32)[:, None].copy(),
        "wl_comb2": cat(inputs["in_Wl1"], inputs["out_Wl1"]),
        "wr_in2": np.asarray(inputs["in_Wr1"], np.float32),
        "wr_out2": np.asarray(inputs["out_Wr1"], np.float32),
        "bias_pk2": np.concatenate(
            [np.asarray(inputs["in_bl1"], np.float32),
             np.asarray(inputs["out_bl1"], np.float32)])[:, None].copy(),
        "wcx2": np.ascontiguousarray(np.asarray(inputs["comb_W1"], np.float32)[0:128]),
        "wch2": np.ascontiguousarray(np.asarray(inputs["comb_W1"], np.float32)[128:256]),
        "cb2": np.asarray(inputs["comb_b1"], np.float32)[:, None].copy(),
        "fw": np.asarray(inputs["final_W"], np.float32).reshape(128, 1).copy(),
        "fb": np.asarray(inputs["final_b"], np.float32).reshape(1, 1).copy(),
    }
    in_maps = []
    for c in range(NC):
        m = dict(common)
        xl = np.zeros((128, NLOCP), dtype=np.float32)
        xl[:, :NLOC] = x.T[:, c * NLOC:(c + 1) * NLOC]
        m["xt_loc"] = xl
        for d in ("in", "out"):
            dinfo = plan["dirs"][d]
            rc = np.zeros((128, NCH), dtype=np.float32)
            r = dinfo["recip"][c]  # [NLOCP]
            rc[:, :] = r.reshape(NCH, 128).T
            m[f"recip_{d}"] = rc.copy()
            for h in ("A", "B"):
                hinfo = dinfo["halves"][h]
                m[f"stream_{d}_{h}"] = _wrap_idx(hinfo["streams"][c])
                m[f"unperm_{d}_{h}"] = _wrap_idx(hinfo["unperm"][c])
        in_maps.append(m)
    return in_maps


def kernel(**inputs):
    plan = _preprocess(np.asarray(inputs["edge_index_in"]),
                       np.asarray(inputs["edge_index_out"]))
    key = tuple(
        (d, h, tuple(plan["dirs"][d]["halves"][h]["NR"]))
        for d in ("in", "out") for h in ("A", "B"))
    if key not in _CACHE:
        _CACHE[key] = _build_program(plan)
    nc = _CACHE[key]
    in_maps = _make_in_maps(plan, inputs)
    res = bass_utils.run_bass_kernel_spmd(nc, in_maps, core_ids=list(range(NC)))
    out = np.concatenate([r["out"][0] for r in res.results])
    return out.astype(np.float32)
